# revision 22
# baseline (speedup 1.0000x reference)
"""Trainium2 Bass kernel for nn_MenuLoss_7713761264358.

Strategy (data parallel over 8 NeuronCores, 64 batch elements each):

Every id lookup in the reference collapses to a row gather data[x, :]
(ids are exact integers after round+mask).  Instead of GPSIMD ap_gather
(~27ns/idx, ~75us), the lookup runs as a bf16 one-hot matmul on the PE:

  1. Ids arrive 16x-replicated per 16-partition group (partition (g, j)
     holds stream g's ids).  True ids come replicated from HBM; pred ids
     come compact [8, L], are round+masked on GPSIMD, and broadcast
     across partitions with a tiny K=8 PE matmul + ACT copy.
  2. DVE/GPSIMD build 14 "id planes" oh_k[(g,j), s] = [x == 16k + j]
     via tensor_scalar is_equal against per-partition iota columns
     (bf16, DVE 4x mode).
  3. PE accumulates 14 matmuls (lhsT_k[16g+j, 8c+g] = packed_table
     [16k+j, c]) into vals[8c+g, s] PSUM — the table lookup for all 16
     packed columns, c-major so the 5 continuous columns live on
     partitions 0:40 and the 7 binary slots on 40:96.  Binary column
     pairs pack as lo + 248*hi ({0,1,248,249} all bf16-exact; batch
     lo-sums <= 168 < 248 so sums unpack exactly).
  4. ACT copies vals to SBUF bf16; DVE multiplies rows 0:40 in place by
     the amounts tile, so ONE food-axis reduce yields nutrition sums
     (rows 0:40) and binary count sums (rows 40:96) together; small
     strided reduces give per-batch / per-day / per-meal sums.
  5. ACT handles tanh/relu/exp/square/abs (penalties, huber, prefs).
  6. All per-batch terms land in one valcat tile, get multiplied by a
     host-built mask-weight tile, and are contracted to a scalar with a
     single ones-column matmul.
Host work is layout-only: de-interleave ids/amounts, replicate/tile
across partitions, pack constant tables, sum 8 per-core partials.
"""

import numpy as np
import ml_dtypes

import concourse.bass as bass
import concourse.tile as tile
from concourse import bacc, mybir

AF = mybir.ActivationFunctionType
OP = mybir.AluOpType
AX = mybir.AxisListType
F32 = mybir.dt.float32
BF16 = mybir.dt.bfloat16
BFNP = ml_dtypes.bfloat16

NCORES = 8
BG = 512            # global batch
BL = BG // NCORES   # 64 batches per core
S = 168             # slots per batch (7 days * 3 meals * 8 foods)
NG = 8              # streams (one per 16-partition group)
NB = BL // NG       # 8 batches per stream
L = NB * S          # 1344 tokens per stream per id-type
L2 = 2 * L          # true tokens | pred tokens
NK = 14             # id planes: 14*16 = 224 >= 223
NKG = 4             # pred id planes built on GPSIMD (k = NK-NKG .. NK-1)
SIG = 248.0         # binary pair packing scale: lo + 248*hi
MAGIC = 8388608.0   # 2^23 round-half-even trick
ZCONST = 3000.0 * 504.0 / 8.0   # per-core constant part of zeros penalty

W_HUB = 1.0 / (100.0 * 512.0)
W_PA = 100.0 / 512.0
NV = 92             # valcat columns (91 used + pad)


def _build(tc, xt, xp8, blk, am8, wts, iotab, mcat, amtb, out):
    import contextlib

    nc = tc.nc

    with contextlib.ExitStack() as ctx:
        sb = ctx.enter_context(tc.tile_pool(name="sb", bufs=1))
        ps = ctx.enter_context(tc.tile_pool(name="ps", bufs=1, space="PSUM"))

        # ---- input DMAs, spread across the 3 usable rings by need-time ----
        # sync ring: iota, true ids (first half), compact pred ids, smalls
        iotab_s = sb.tile([128, NK], F32, tag="iotab_s")
        nc.sync.dma_start(out=iotab_s[:], in_=iotab)
        xcat = sb.tile([128, L2], BF16, tag="xcat")
        nc.sync.dma_start(out=xcat[:, 0:L // 2], in_=xt[:, 0:L // 2])
        xp8_s = sb.tile([8, L], F32, tag="xp8_s")
        nc.sync.dma_start(out=xp8_s[:], in_=xp8)
        blk_s = sb.tile([8, 128], BF16, tag="blk_s")
        nc.sync.dma_start(out=blk_s[:], in_=blk)
        am8_s = sb.tile([8, L], F32, tag="am8_s")
        nc.sync.dma_start(out=am8_s[:], in_=am8)
        mcat_s = sb.tile([128, NV], F32, tag="mcat_s")
        nc.sync.dma_start(out=mcat_s[:], in_=mcat)
        # scalar ring: true ids (second half), weights by k-need, amounts
        nc.scalar.dma_start(out=xcat[:, L // 2:L], in_=xt[:, L // 2:L])
        wts_s = sb.tile([128, NK * 128 + 1], BF16, tag="wts_s")
        nc.scalar.dma_start(out=wts_s[:, 0:3 * 128], in_=wts[:, 0:3 * 128])
        nc.scalar.dma_start(
            out=wts_s[:, 3 * 128:8 * 128], in_=wts[:, 3 * 128:8 * 128])
        nc.scalar.dma_start(
            out=wts_s[:, 8 * 128:], in_=wts[:, 8 * 128:])
        amtb_s = sb.tile([40, L2], BF16, tag="amtb_s")
        nc.scalar.dma_start(out=amtb_s[:], in_=amtb)

        # gpsimd queue: constants via memset, then pred-id preprocessing
        valcat = sb.tile([128, NV], F32, tag="valcat")
        nc.gpsimd.memset(valcat[:], 0.0)
        cm222 = sb.tile([128, 1], F32, tag="cm222")
        nc.gpsimd.memset(cm222[:], -222.0)
        cm1680 = sb.tile([128, 1], F32, tag="cm1680")
        nc.gpsimd.memset(cm1680[:], -1680.0)
        ones_t = sb.tile([128, 1], F32, tag="ones_t")
        nc.gpsimd.memset(ones_t[:], 1.0)
        d1 = sb.tile([40, 32], F32, tag="d1")
        nc.gpsimd.memset(d1[:], 0.0)

        # pred ids on GPSIMD (immediates only): round-half-even, then the
        # >222.5 mask becomes clamp-to-223 (table row 223 = data row 0)
        kp8 = sb.tile([8, L], F32, tag="kp8")
        nc.gpsimd.tensor_scalar(
            out=kp8[:], in0=xp8_s[:], scalar1=MAGIC, scalar2=MAGIC,
            op0=OP.add, op1=OP.subtract,
        )
        rp8 = sb.tile([8, L], BF16, tag="rp8")
        nc.gpsimd.tensor_scalar(
            out=rp8[:], in0=kp8[:], scalar1=223.0, scalar2=None, op0=OP.min,
        )

        # ---- DVE: true-id planes (k = 0..13) ----
        oh = [
            sb.tile([128, L2], BF16, name=f"oh{k}", tag=f"oh{k}")
            for k in range(NK)
        ]
        for k in range(NK):
            nc.vector.tensor_scalar(
                out=oh[k][:, 0:L], in0=xcat[:, 0:L],
                scalar1=iotab_s[:, k:k + 1], scalar2=None, op0=OP.is_equal,
            )

        # ---- PE: true-half lookup matmuls, k outer (LDW amortized) ----
        vals_t = ps.tile([128, 1536], F32, tag="vals_t")
        vals_p = ps.tile([128, 1536], F32, tag="vals_p")

        def lookup_mms(v_t, h, k):
            for c0 in (0, 512, 1024):
                w = min(512, L - c0)
                nc.tensor.matmul(
                    v_t[:, c0:c0 + w],
                    wts_s[:, 128 * k:128 * (k + 1)],
                    oh[k][:, h * L + c0:h * L + c0 + w],
                    start=(k == 0), stop=(k == NK - 1),
                )

        for k in range(3):
            lookup_mms(vals_t, 0, k)
        # broadcast rounded pred ids across partitions (K=8, into vals_p)
        for c0 in (0, 512, 1024):
            w = min(512, L - c0)
            nc.tensor.matmul(
                vals_p[:, c0:c0 + w], blk_s[:], rp8[:, c0:c0 + w],
                start=True, stop=True,
            )
        for k in range(3, NK):
            lookup_mms(vals_t, 0, k)

        # ---- ACT: penalties + pred-id broadcast copy ----
        th1 = sb.tile([8, L], F32, tag="th1")
        nc.scalar.activation(
            out=th1[:], in_=xp8_s[:], func=AF.Tanh, scale=2.0,
            accum_out=valcat[0:8, 88:89],
        )
        nc.scalar.activation(
            out=xcat[:, L:L2], in_=vals_p[:, 0:L], func=AF.Copy, scale=1.0,
        )
        rl1 = sb.tile([8, L], F32, tag="rl1")
        nc.scalar.activation(
            out=rl1[:], in_=xp8_s[:], func=AF.Relu, bias=cm222[0:8, :],
            scale=1.0, accum_out=valcat[0:8, 89:90],
        )
        th2 = sb.tile([8, L], F32, tag="th2")
        nc.scalar.activation(
            out=th2[:], in_=am8_s[:], func=AF.Tanh, scale=2.0,
            accum_out=valcat[0:8, 90:91],
        )

        # ---- pred-id planes: DVE k=0..9 (ptr iota), GPSIMD k=10..13 ----
        # Pool only takes immediate scalars: pre-subtract j = p%16 on DVE,
        # then each GPSIMD plane is an is_eq against the immediate 16k.
        zp = sb.tile([128, L], BF16, tag="zp")
        nc.vector.tensor_scalar(
            out=zp[:], in0=xcat[:, L:L2],
            scalar1=iotab_s[:, 0:1], scalar2=None, op0=OP.subtract,
        )
        for k in range(NK - NKG, NK):
            nc.gpsimd.tensor_scalar(
                out=oh[k][:, L:L2], in0=zp[:],
                scalar1=float(16 * k), scalar2=None, op0=OP.is_equal,
            )
        for k in range(NK - NKG):
            nc.vector.tensor_scalar(
                out=oh[k][:, L:L2], in0=xcat[:, L:L2],
                scalar1=iotab_s[:, k:k + 1], scalar2=None, op0=OP.is_equal,
            )

        # ---- PE: pred-half lookup matmuls ----
        for k in range(NK):
            lookup_mms(vals_p, 1, k)

        # ---- per half: ACT copy PSUM->SBUF bf16; in-place amounts
        # product on cont rows 0:40; fused food-axis reduce rows 0:96 ----
        vals_sb = sb.tile([128, L2], BF16, tag="vals_sb")
        prd8 = sb.tile([120, 336], F32, tag="prd8")  # (h, b, d, m)

        def half_tail(h, v_t, chunks):
            for a0, a1 in chunks:
                cs = slice(h * L + a0, h * L + a1)
                nc.scalar.activation(
                    out=vals_sb[:, cs], in_=v_t[:, a0:a1],
                    func=AF.Copy, scale=1.0,
                )
                nc.vector.tensor_tensor(
                    out=vals_sb[0:40, cs], in0=vals_sb[0:40, cs],
                    in1=amtb_s[:, cs], op=OP.mult,
                )
                nc.vector.tensor_reduce(
                    out=prd8[:, (h * L + a0) // 8:(h * L + a1) // 8],
                    in_=vals_sb[0:120, cs].rearrange("p (u f) -> p u f", f=8),
                    axis=AX.X, op=OP.add,
                )

        half_tail(0, vals_t, [(0, L)])
        half_tail(1, vals_p, [(0, 672), (672, L)])

        # ---- second-stage reduces ----
        psums = sb.tile([120, 16], F32, tag="psums")  # (h, b)
        nc.vector.tensor_reduce(
            out=psums[:], in_=prd8[:].rearrange("p (hb u) -> p hb u", u=21),
            axis=AX.X, op=OP.add,
        )
        meal = sb.tile([8, 48], F32, tag="meal")     # (h, b, m)
        nc.vector.tensor_reduce(
            out=meal[:].rearrange("p (hb m) -> p hb m", m=3),
            in_=prd8[0:8, :].rearrange("p (hb d m) -> p hb m d", d=7, m=3),
            axis=AX.X, op=OP.add,
        )
        day = sb.tile([8, 56], F32, tag="day")       # (b, d) pred half
        nc.vector.tensor_reduce(
            out=day[:].rearrange("p (b d) -> p b d", d=7),
            in_=prd8[0:8, 168:336].rearrange("p (b d m) -> p b d m", d=7, m=3),
            axis=AX.X, op=OP.add,
        )

        # ---- day-level variance: var = s2/7 - (s1/700)^2, cal = day/100 ----
        sq = sb.tile([8, 56], F32, tag="sq")
        nc.scalar.activation(out=sq[:], in_=day[:], func=AF.Square, scale=0.01)
        s2 = sb.tile([8, 8], F32, tag="s2")
        nc.vector.tensor_reduce(
            out=s2[:], in_=sq[:].rearrange("p (b d) -> p b d", d=7),
            axis=AX.X, op=OP.add,
        )
        mu2 = sb.tile([8, 8], F32, tag="mu2")
        nc.vector.scalar_tensor_tensor(
            out=mu2[:], in0=psums[0:8, 8:16], scalar=1.0 / 490000.0,
            in1=psums[0:8, 8:16], op0=OP.mult, op1=OP.mult,
        )
        nc.vector.scalar_tensor_tensor(
            out=valcat[0:8, 80:88], in0=s2[:], scalar=1.0 / 7.0, in1=mu2[:],
            op0=OP.mult, op1=OP.subtract,
        )

        # ---- unpack binary sums (rows 64:120): S = lo + 248*hi ----
        # All tiles full-height, sliced at base partition 64 (walrus
        # requires equal SBUF base partitions across operands).
        # PG cols: 0:8 lot, 8:16 hit, 16:24 lop, 24:32 hip
        B = slice(64, 120)
        bs = psums[B, :]
        pg = sb.tile([128, 32], F32, tag="pg")
        t1 = sb.tile([128, 16], F32, tag="t1")
        nc.vector.tensor_scalar(
            out=t1[B, :], in0=bs, scalar1=1.0 / SIG,
            scalar2=MAGIC - 0.33871, op0=OP.mult, op1=OP.add,
        )
        hi_v = pg[B, :].rearrange("p (v q b) -> p v q b", v=2, q=2)[:, :, 1:2, :]
        lo_v = pg[B, :].rearrange("p (v q b) -> p v q b", v=2, q=2)[:, :, 0:1, :]
        nc.vector.tensor_scalar(
            out=hi_v, in0=t1[B, :], scalar1=MAGIC, scalar2=None,
            op0=OP.subtract,
        )
        nc.vector.scalar_tensor_tensor(
            out=lo_v, in0=hi_v, scalar=-SIG, in1=bs,
            op0=OP.mult, op1=OP.add,
        )
        g2 = pg[B, 0:16]   # gold (true):  lot | hit
        p2 = pg[B, 16:32]  # pred:         lop | hip

        # ---- huber terms ----
        nc.vector.tensor_tensor(
            out=d1[:, 0:8], in0=psums[0:40, 8:16], in1=psums[0:40, 0:8],
            op=OP.subtract,
        )
        nc.vector.tensor_tensor(
            out=d1[0:8, 8:32], in0=meal[:, 24:48], in1=meal[:, 0:24],
            op=OP.subtract,
        )
        d2 = sb.tile([128, 16], F32, tag="d2")
        nc.vector.tensor_tensor(out=d2[B, :], in0=p2, in1=g2, op=OP.subtract)

        def huber(dst, d_ap, scale, psl, w, tag):
            a_t = sb.tile([128, w], F32, tag=tag + "_a")
            nc.scalar.activation(
                out=a_t[psl, :], in_=d_ap, func=AF.Abs, scale=scale)
            m_t = sb.tile([128, w], F32, tag=tag + "_m")
            nc.vector.tensor_scalar(
                out=m_t[psl, :], in0=a_t[psl, :], scalar1=1.0, scalar2=None,
                op0=OP.min,
            )
            t_t = sb.tile([128, w], F32, tag=tag + "_t")
            nc.vector.scalar_tensor_tensor(
                out=t_t[psl, :], in0=m_t[psl, :], scalar=-0.5,
                in1=a_t[psl, :], op0=OP.mult, op1=OP.add,
            )
            nc.vector.tensor_tensor(
                out=dst, in0=m_t[psl, :], in1=t_t[psl, :], op=OP.mult)

        huber(valcat[0:40, 0:32], d1[:], 1.0 / 700.0, slice(0, 40), 32, "h1")
        huber(valcat[B, 32:48], d2[B, :], 1.0, B, 16, "h2")

        # ---- pref/allergen terms -> valcat[64:120, 48:80] ----
        gc = sb.tile([128, 16], F32, tag="gc")
        nc.vector.tensor_scalar(
            out=gc[B, :], in0=g2, scalar1=168.0, scalar2=None, op0=OP.min
        )
        e1 = sb.tile([128, 16], F32, tag="e1")
        nc.scalar.activation(
            out=e1[B, :], in_=gc[B, :], func=AF.Exp, scale=10.0,
            bias=cm1680[B, :],
        )
        p1 = sb.tile([128, 16], F32, tag="p1")
        nc.vector.tensor_scalar(
            out=p1[B, :], in0=p2, scalar1=-1.0, scalar2=168.0,
            op0=OP.mult, op1=OP.add,
        )
        q1 = sb.tile([128, 16], F32, tag="q1")
        nc.scalar.activation(out=q1[B, :], in_=p1[B, :], func=AF.Square)
        nc.vector.tensor_tensor(
            out=valcat[B, 48:64], in0=e1[B, :], in1=q1[B, :], op=OP.mult
        )
        gp = sb.tile([128, 16], F32, tag="gp")
        nc.vector.tensor_scalar(
            out=gp[B, :], in0=g2, scalar1=0.0, scalar2=None, op0=OP.max
        )
        e2 = sb.tile([128, 16], F32, tag="e2")
        nc.scalar.activation(out=e2[B, :], in_=gp[B, :], func=AF.Exp, scale=-10.0)
        q2 = sb.tile([128, 16], F32, tag="q2")
        nc.scalar.activation(out=q2[B, :], in_=p2, func=AF.Square)
        nc.vector.tensor_tensor(
            out=valcat[B, 64:80], in0=e2[B, :], in1=q2[B, :], op=OP.mult
        )

        # ---- weighted contraction: one mult + one ones-column matmul ----
        wv = sb.tile([128, NV], F32, tag="wv")
        nc.vector.tensor_tensor(
            out=wv[:], in0=valcat[:], in1=mcat_s[:], op=OP.mult
        )
        fps = ps.tile([1, NV], F32, tag="fps")
        nc.tensor.matmul(
            fps[:], ones_t[:], wv[:], start=True, stop=True,
        )
        loss_t = sb.tile([1, 1], F32, tag="loss_t")
        nc.vector.tensor_reduce(out=loss_t[:], in_=fps[:], axis=AX.X, op=OP.add)
        lossf = sb.tile([1, 1], F32, tag="lossf")
        nc.vector.tensor_scalar_add(out=lossf[:], in0=loss_t[:], scalar1=ZCONST)
        nc.sync.dma_start(out=out, in_=lossf[:])


def build_program(ndev=NCORES):
    nc = bacc.Bacc("TRN2", target_bir_lowering=False, num_devices=ndev)
    xt = nc.dram_tensor("xt", [128, L], BF16, kind="ExternalInput")
    xp8 = nc.dram_tensor("xp8", [8, L], F32, kind="ExternalInput")
    blk = nc.dram_tensor("blk", [8, 128], BF16, kind="ExternalInput")
    am8 = nc.dram_tensor("am8", [8, L], F32, kind="ExternalInput")
    wts = nc.dram_tensor("wts", [128, NK * 128 + 1], BF16, kind="ExternalInput")
    iotab = nc.dram_tensor("iotab", [128, NK], F32, kind="ExternalInput")
    mcat = nc.dram_tensor("mcat", [128, NV], F32, kind="ExternalInput")
    amtb = nc.dram_tensor("amtb", [40, L2], BF16, kind="ExternalInput")
    out = nc.dram_tensor("o", [1, 1], F32, kind="ExternalOutput")
    with tile.TileContext(nc) as tc:
        _build(
            tc, xt.ap(), xp8.ap(), blk.ap(), am8.ap(), wts.ap(),
            iotab.ap(), mcat.ap(), amtb.ap(), out.ap(),
        )
    nc.compile()
    return nc


def make_const_inputs(data):
    """Host-side constant tables shared by all cores (layout only)."""
    data = np.asarray(data, dtype=np.float32)
    # packed table [224, 16]: 5 cont cols, 7 sigma-packed binary pairs
    pk = np.zeros((224, 16), np.float32)
    pk[:223, 0:5] = data[:, 0:5]
    for u in range(7):
        pk[:223, 5 + u] = data[:, 5 + 2 * u] + SIG * data[:, 6 + 2 * u]
    pk[223, :] = pk[0, :]   # clamp target: id 223 behaves like id 0
    # 14 lhsT planes: lhsT_k[16g+j, 8c+g] = pk[16k+j, c]  (c-major out)
    wts = np.zeros((128, NK * 128 + 1), BFNP)
    for k in range(NK):
        blkm = np.zeros((128, 128), np.float32)
        for g in range(NG):
            for c in range(5):
                blkm[16 * g:16 * g + 16, 8 * c + g] = pk[16 * k:16 * k + 16, c]
            for u in range(7):
                blkm[16 * g:16 * g + 16, 64 + 8 * u + g] = (
                    pk[16 * k:16 * k + 16, 5 + u])
        wts[:, 128 * k:128 * (k + 1)] = blkm.astype(BFNP)
    wts[:, NK * 128] = np.float32(1.0)
    # iota planes: value 16k + (p % 16)
    iotab = (
        16.0 * np.arange(NK)[None, :] + (np.arange(128) % 16)[:, None]
    ).astype(np.float32)
    # broadcast selector: blk[g, p] = (p // 16 == g)  (match (g, j) layout)
    blk = (np.arange(128)[None, :] // 16 == np.arange(8)[:, None]).astype(BFNP)
    # mask-weight tile [128, NV]; c-major: row r -> c = r // 8
    r = np.arange(128)
    c = r // 8
    u = (r - 64) // 8  # binary slot index for rows 64:120
    m = np.zeros((128, NV), np.float32)
    m[:, 0:8] = (((r < 40) & (c < 5)) * W_HUB)[:, None]        # nutrition
    m[:, 8:32] = ((r < 8) * W_HUB)[:, None]                    # meal huber
    binrow = (r >= 64) & (r < 120)
    m[:, 32:40] = ((binrow & ((u == 5) | (u == 6))) * W_HUB)[:, None]
    m[:, 40:48] = ((binrow & (u >= 4) & (u <= 6)) * W_HUB)[:, None]
    m[:, 48:64] = ((binrow & (u == 0)) * W_PA)[:, None]        # prefs
    m[:, 64:72] = ((binrow & (u >= 1) & (u <= 4)) * W_PA)[:, None]
    m[:, 72:80] = ((binrow & (u >= 1) & (u <= 3)) * W_PA)[:, None]
    m[:, 80:88] = ((r < 8) / 512.0)[:, None]           # day variance
    m[0:8, 88] = -2.0 * 3000.0 / 512.0                 # tanh(2*pid) compact
    m[8:, 88] = 0.0
    m[0:8, 89] = 1.0 / 512.0                           # relu(pid-222) compact
    m[0:8, 90] = -3000.0 / 512.0                       # tanh(2*pamt) compact
    return wts, iotab, blk, m


def make_in_maps(y_pred, y, data):
    y_pred = np.asarray(y_pred, dtype=np.float32)
    y = np.asarray(y, dtype=np.float32)
    wts, iotab, blk, mcat = make_const_inputs(data)
    in_maps = []
    for core in range(NCORES):
        sl = slice(core * BL, (core + 1) * BL)

        def streams(arr, comp):
            # [64, 7, 3, 8] -> [8 streams, 1344] (batch-major within stream)
            return np.ascontiguousarray(
                arr[sl, ..., comp], dtype=np.float32).reshape(NG, L)

        pid = streams(y_pred, 0)
        pam = streams(y_pred, 1)
        tid = streams(y, 0)
        tam = streams(y, 1)
        xt = np.repeat(tid.astype(BFNP), 16, axis=0)         # [128, L] bf16
        # amounts rows 8c+g (c<5): amounts stream g; true | pred
        amtb = np.tile(
            np.concatenate([tam, pam], axis=1), (5, 1)).astype(BFNP)
        in_maps.append({
            "xt": xt, "xp8": pid, "blk": blk, "am8": pam,
            "wts": wts, "iotab": iotab, "mcat": mcat, "amtb": amtb,
        })
    return in_maps


_NC_CACHE = None


def _get_nc():
    global _NC_CACHE
    if _NC_CACHE is None:
        _NC_CACHE = build_program()
    return _NC_CACHE


def run_on_hw(y_pred, y, data, **kwargs):
    from concourse.bass_utils import run_bass_kernel_spmd

    nc = _get_nc()
    in_maps = make_in_maps(y_pred, y, data)
    res = run_bass_kernel_spmd(
        nc, in_maps, core_ids=list(range(NCORES)), **kwargs
    )
    parts = [r["o"][0, 0] for r in res.results]
    return np.float32(np.sum(np.asarray(parts, dtype=np.float32))), res


def kernel(y_pred, y, data):
    return run_on_hw(y_pred, y, data)[0]


# revision 24
# speedup vs baseline: 3.4583x; 3.4583x over previous
"""Trainium2 Bass kernel for nn_MenuLoss_7713761264358.

Strategy (data parallel over 8 NeuronCores, 64 batch elements each):

Every id lookup in the reference collapses to a row gather data[x, :]
(ids are exact integers after round+mask).  Instead of GPSIMD ap_gather
(~27ns/idx, ~75us), the lookup runs as a bf16 one-hot matmul on the PE:

  1. Ids arrive 16x-replicated per 16-partition group (partition (g, j)
     holds stream g's ids).  True ids come replicated from HBM; pred ids
     come compact [8, L], are round+masked on GPSIMD, and broadcast
     across partitions with a tiny K=8 PE matmul + ACT copy.
  2. DVE/GPSIMD build 14 "id planes" oh_k[(g,j), s] = [x == 16k + j]
     via tensor_scalar is_equal against per-partition iota columns
     (bf16, DVE 4x mode).
  3. PE accumulates 14 matmuls (lhsT_k[16g+j, 8c+g] = packed_table
     [16k+j, c]) into vals[8c+g, s] PSUM — the table lookup for all 16
     packed columns, c-major so the 5 continuous columns live on
     partitions 0:40 and the 7 binary slots on 40:96.  Binary column
     pairs pack as lo + 248*hi ({0,1,248,249} all bf16-exact; batch
     lo-sums <= 168 < 248 so sums unpack exactly).
  4. ACT copies vals to SBUF bf16; DVE multiplies rows 0:40 in place by
     the amounts tile, so ONE food-axis reduce yields nutrition sums
     (rows 0:40) and binary count sums (rows 40:96) together; small
     strided reduces give per-batch / per-day / per-meal sums.
  5. ACT handles tanh/relu/exp/square/abs (penalties, huber, prefs).
  6. All per-batch terms land in one valcat tile, get multiplied by a
     host-built mask-weight tile, and are contracted to a scalar with a
     single ones-column matmul.
Host work is layout-only: de-interleave ids/amounts, replicate/tile
across partitions, pack constant tables, sum 8 per-core partials.
"""

import numpy as np
import ml_dtypes

import concourse.bass as bass
import concourse.tile as tile
from concourse import bacc, mybir

AF = mybir.ActivationFunctionType
OP = mybir.AluOpType
AX = mybir.AxisListType
F32 = mybir.dt.float32
BF16 = mybir.dt.bfloat16
BFNP = ml_dtypes.bfloat16

NCORES = 8
BG = 512            # global batch
BL = BG // NCORES   # 64 batches per core
S = 168             # slots per batch (7 days * 3 meals * 8 foods)
NG = 8              # streams (one per 16-partition group)
NB = BL // NG       # 8 batches per stream
L = NB * S          # 1344 tokens per stream per id-type
L2 = 2 * L          # true tokens | pred tokens
NK = 14             # id planes: 14*16 = 224 >= 223
NKG = 4             # pred id planes built on GPSIMD (k = NK-NKG .. NK-1)
SIG = 248.0         # binary pair packing scale: lo + 248*hi
MAGIC = 8388608.0   # 2^23 round-half-even trick
ZCONST = 3000.0 * 504.0 / 8.0   # per-core constant part of zeros penalty

W_HUB = 1.0 / (100.0 * 512.0)
W_PA = 100.0 / 512.0
NV = 92             # valcat columns (91 used + pad)


def _build(tc, xt, xp8, blk, am8, wts, iotab, mcat, amtb, out):
    import contextlib

    nc = tc.nc

    with contextlib.ExitStack() as ctx:
        sb = ctx.enter_context(tc.tile_pool(name="sb", bufs=1))
        ps = ctx.enter_context(tc.tile_pool(name="ps", bufs=1, space="PSUM"))

        # ---- input DMAs, spread across the 3 usable rings by need-time ----
        # sync ring: iota, true ids (first half), compact pred ids, smalls
        iotab_s = sb.tile([128, NK], F32, tag="iotab_s")
        nc.sync.dma_start(out=iotab_s[:], in_=iotab)
        xcat = sb.tile([128, L2], BF16, tag="xcat")
        nc.sync.dma_start(out=xcat[:, 0:L // 2], in_=xt[:, 0:L // 2])
        xp8_s = sb.tile([8, L], F32, tag="xp8_s")
        nc.sync.dma_start(out=xp8_s[:], in_=xp8)
        blk_s = sb.tile([8, 128], BF16, tag="blk_s")
        nc.sync.dma_start(out=blk_s[:], in_=blk)
        am8_s = sb.tile([8, L], F32, tag="am8_s")
        nc.sync.dma_start(out=am8_s[:], in_=am8)
        mcat_s = sb.tile([128, NV], F32, tag="mcat_s")
        nc.sync.dma_start(out=mcat_s[:], in_=mcat)
        # scalar ring: true ids (second half), weights by k-need, amounts
        nc.scalar.dma_start(out=xcat[:, L // 2:L], in_=xt[:, L // 2:L])
        wts_s = sb.tile([128, NK * 128 + 1], BF16, tag="wts_s")
        nc.scalar.dma_start(out=wts_s[:, 0:3 * 128], in_=wts[:, 0:3 * 128])
        nc.scalar.dma_start(
            out=wts_s[:, 3 * 128:8 * 128], in_=wts[:, 3 * 128:8 * 128])
        nc.scalar.dma_start(
            out=wts_s[:, 8 * 128:], in_=wts[:, 8 * 128:])
        amtb_s = sb.tile([40, L2], BF16, tag="amtb_s")
        nc.scalar.dma_start(out=amtb_s[:], in_=amtb)

        # gpsimd queue: constants via memset, then pred-id preprocessing
        valcat = sb.tile([128, NV], F32, tag="valcat")
        nc.gpsimd.memset(valcat[:], 0.0)
        cm222 = sb.tile([128, 1], F32, tag="cm222")
        nc.gpsimd.memset(cm222[:], -222.0)
        cm1680 = sb.tile([128, 1], F32, tag="cm1680")
        nc.gpsimd.memset(cm1680[:], -1680.0)
        ones_t = sb.tile([128, 1], F32, tag="ones_t")
        nc.gpsimd.memset(ones_t[:], 1.0)
        d1 = sb.tile([40, 32], F32, tag="d1")
        nc.gpsimd.memset(d1[:], 0.0)

        # ---- DVE: true-id planes; pred round+clamp interleaved early.
        # (GPSIMD tensor ops measured ~15ns/elem on HW and stall DVE via
        # the shared SBUF ports — keep all bulk elementwise on DVE.)
        oh = [
            sb.tile([128, L2], BF16, name=f"oh{k}", tag=f"oh{k}")
            for k in range(NK)
        ]
        for k in range(2):
            nc.vector.tensor_scalar(
                out=oh[k][:, 0:L], in0=xcat[:, 0:L],
                scalar1=iotab_s[:, k:k + 1], scalar2=None, op0=OP.is_equal,
            )
        # pred ids: round-half-even, then the >222.5 mask becomes
        # clamp-to-223 (table row 223 = data row 0)
        kp8 = sb.tile([8, L], F32, tag="kp8")
        nc.vector.tensor_scalar(
            out=kp8[:], in0=xp8_s[:], scalar1=MAGIC, scalar2=MAGIC,
            op0=OP.add, op1=OP.subtract,
        )
        rp8 = sb.tile([8, L], BF16, tag="rp8")
        nc.vector.tensor_scalar(
            out=rp8[:], in0=kp8[:], scalar1=223.0, scalar2=None, op0=OP.min,
        )
        for k in range(2, NK):
            nc.vector.tensor_scalar(
                out=oh[k][:, 0:L], in0=xcat[:, 0:L],
                scalar1=iotab_s[:, k:k + 1], scalar2=None, op0=OP.is_equal,
            )

        # ---- PE: true-half lookup matmuls, k outer (LDW amortized) ----
        vals_t = ps.tile([128, 1536], F32, tag="vals_t")
        vals_p = ps.tile([128, 1536], F32, tag="vals_p")

        def lookup_mms(v_t, h, k):
            for c0 in (0, 512, 1024):
                w = min(512, L - c0)
                nc.tensor.matmul(
                    v_t[:, c0:c0 + w],
                    wts_s[:, 128 * k:128 * (k + 1)],
                    oh[k][:, h * L + c0:h * L + c0 + w],
                    start=(k == 0), stop=(k == NK - 1),
                )

        for k in range(3):
            lookup_mms(vals_t, 0, k)
        # broadcast rounded pred ids across partitions (K=8, into vals_p)
        for c0 in (0, 512, 1024):
            w = min(512, L - c0)
            nc.tensor.matmul(
                vals_p[:, c0:c0 + w], blk_s[:], rp8[:, c0:c0 + w],
                start=True, stop=True,
            )
        for k in range(3, NK):
            lookup_mms(vals_t, 0, k)

        # ---- ACT: penalties + pred-id broadcast copy ----
        th1 = sb.tile([8, L], F32, tag="th1")
        nc.scalar.activation(
            out=th1[:], in_=xp8_s[:], func=AF.Tanh, scale=2.0,
            accum_out=valcat[0:8, 88:89],
        )
        nc.scalar.activation(
            out=xcat[:, L:L2], in_=vals_p[:, 0:L], func=AF.Copy, scale=1.0,
        )
        rl1 = sb.tile([8, L], F32, tag="rl1")
        nc.scalar.activation(
            out=rl1[:], in_=xp8_s[:], func=AF.Relu, bias=cm222[0:8, :],
            scale=1.0, accum_out=valcat[0:8, 89:90],
        )
        th2 = sb.tile([8, L], F32, tag="th2")
        nc.scalar.activation(
            out=th2[:], in_=am8_s[:], func=AF.Tanh, scale=2.0,
            accum_out=valcat[0:8, 90:91],
        )

        # ---- pred-id planes (DVE) ----
        for k in range(NK):
            nc.vector.tensor_scalar(
                out=oh[k][:, L:L2], in0=xcat[:, L:L2],
                scalar1=iotab_s[:, k:k + 1], scalar2=None, op0=OP.is_equal,
            )

        # ---- PE: pred-half lookup matmuls ----
        for k in range(NK):
            lookup_mms(vals_p, 1, k)

        # ---- per half: ACT copy PSUM->SBUF bf16; in-place amounts
        # product on cont rows 0:40; fused food-axis reduce rows 0:96 ----
        vals_sb = sb.tile([128, L2], BF16, tag="vals_sb")
        prd8 = sb.tile([120, 336], F32, tag="prd8")  # (h, b, d, m)

        def half_tail(h, v_t, chunks):
            for a0, a1 in chunks:
                cs = slice(h * L + a0, h * L + a1)
                nc.scalar.activation(
                    out=vals_sb[:, cs], in_=v_t[:, a0:a1],
                    func=AF.Copy, scale=1.0,
                )
                nc.vector.tensor_tensor(
                    out=vals_sb[0:40, cs], in0=vals_sb[0:40, cs],
                    in1=amtb_s[:, cs], op=OP.mult,
                )
                nc.vector.tensor_reduce(
                    out=prd8[:, (h * L + a0) // 8:(h * L + a1) // 8],
                    in_=vals_sb[0:120, cs].rearrange("p (u f) -> p u f", f=8),
                    axis=AX.X, op=OP.add,
                )

        half_tail(0, vals_t, [(0, L)])
        half_tail(1, vals_p, [(0, 672), (672, L)])

        # ---- second-stage reduces ----
        psums = sb.tile([120, 16], F32, tag="psums")  # (h, b)
        nc.vector.tensor_reduce(
            out=psums[:], in_=prd8[:].rearrange("p (hb u) -> p hb u", u=21),
            axis=AX.X, op=OP.add,
        )
        meal = sb.tile([8, 48], F32, tag="meal")     # (h, b, m)
        nc.vector.tensor_reduce(
            out=meal[:].rearrange("p (hb m) -> p hb m", m=3),
            in_=prd8[0:8, :].rearrange("p (hb d m) -> p hb m d", d=7, m=3),
            axis=AX.X, op=OP.add,
        )
        day = sb.tile([8, 56], F32, tag="day")       # (b, d) pred half
        nc.vector.tensor_reduce(
            out=day[:].rearrange("p (b d) -> p b d", d=7),
            in_=prd8[0:8, 168:336].rearrange("p (b d m) -> p b d m", d=7, m=3),
            axis=AX.X, op=OP.add,
        )

        # ---- day-level variance: var = s2/7 - (s1/700)^2, cal = day/100 ----
        sq = sb.tile([8, 56], F32, tag="sq")
        nc.scalar.activation(out=sq[:], in_=day[:], func=AF.Square, scale=0.01)
        s2 = sb.tile([8, 8], F32, tag="s2")
        nc.vector.tensor_reduce(
            out=s2[:], in_=sq[:].rearrange("p (b d) -> p b d", d=7),
            axis=AX.X, op=OP.add,
        )
        mu2 = sb.tile([8, 8], F32, tag="mu2")
        nc.vector.scalar_tensor_tensor(
            out=mu2[:], in0=psums[0:8, 8:16], scalar=1.0 / 490000.0,
            in1=psums[0:8, 8:16], op0=OP.mult, op1=OP.mult,
        )
        nc.vector.scalar_tensor_tensor(
            out=valcat[0:8, 80:88], in0=s2[:], scalar=1.0 / 7.0, in1=mu2[:],
            op0=OP.mult, op1=OP.subtract,
        )

        # ---- unpack binary sums (rows 64:120): S = lo + 248*hi ----
        # All tiles full-height, sliced at base partition 64 (walrus
        # requires equal SBUF base partitions across operands).
        # PG cols: 0:8 lot, 8:16 hit, 16:24 lop, 24:32 hip
        B = slice(64, 120)
        bs = psums[B, :]
        pg = sb.tile([128, 32], F32, tag="pg")
        t1 = sb.tile([128, 16], F32, tag="t1")
        nc.vector.tensor_scalar(
            out=t1[B, :], in0=bs, scalar1=1.0 / SIG,
            scalar2=MAGIC - 0.33871, op0=OP.mult, op1=OP.add,
        )
        hi_v = pg[B, :].rearrange("p (v q b) -> p v q b", v=2, q=2)[:, :, 1:2, :]
        lo_v = pg[B, :].rearrange("p (v q b) -> p v q b", v=2, q=2)[:, :, 0:1, :]
        nc.vector.tensor_scalar(
            out=hi_v, in0=t1[B, :], scalar1=MAGIC, scalar2=None,
            op0=OP.subtract,
        )
        nc.vector.scalar_tensor_tensor(
            out=lo_v, in0=hi_v, scalar=-SIG, in1=bs,
            op0=OP.mult, op1=OP.add,
        )
        g2 = pg[B, 0:16]   # gold (true):  lot | hit
        p2 = pg[B, 16:32]  # pred:         lop | hip

        # ---- huber terms ----
        nc.vector.tensor_tensor(
            out=d1[:, 0:8], in0=psums[0:40, 8:16], in1=psums[0:40, 0:8],
            op=OP.subtract,
        )
        nc.vector.tensor_tensor(
            out=d1[0:8, 8:32], in0=meal[:, 24:48], in1=meal[:, 0:24],
            op=OP.subtract,
        )
        d2 = sb.tile([128, 16], F32, tag="d2")
        nc.vector.tensor_tensor(out=d2[B, :], in0=p2, in1=g2, op=OP.subtract)

        def huber(dst, d_ap, scale, psl, w, tag):
            a_t = sb.tile([128, w], F32, tag=tag + "_a")
            nc.scalar.activation(
                out=a_t[psl, :], in_=d_ap, func=AF.Abs, scale=scale)
            m_t = sb.tile([128, w], F32, tag=tag + "_m")
            nc.vector.tensor_scalar(
                out=m_t[psl, :], in0=a_t[psl, :], scalar1=1.0, scalar2=None,
                op0=OP.min,
            )
            t_t = sb.tile([128, w], F32, tag=tag + "_t")
            nc.vector.scalar_tensor_tensor(
                out=t_t[psl, :], in0=m_t[psl, :], scalar=-0.5,
                in1=a_t[psl, :], op0=OP.mult, op1=OP.add,
            )
            nc.vector.tensor_tensor(
                out=dst, in0=m_t[psl, :], in1=t_t[psl, :], op=OP.mult)

        huber(valcat[0:40, 0:32], d1[:], 1.0 / 700.0, slice(0, 40), 32, "h1")
        huber(valcat[B, 32:48], d2[B, :], 1.0, B, 16, "h2")

        # ---- pref/allergen terms -> valcat[64:120, 48:80] ----
        gc = sb.tile([128, 16], F32, tag="gc")
        nc.vector.tensor_scalar(
            out=gc[B, :], in0=g2, scalar1=168.0, scalar2=None, op0=OP.min
        )
        e1 = sb.tile([128, 16], F32, tag="e1")
        nc.scalar.activation(
            out=e1[B, :], in_=gc[B, :], func=AF.Exp, scale=10.0,
            bias=cm1680[B, :],
        )
        p1 = sb.tile([128, 16], F32, tag="p1")
        nc.vector.tensor_scalar(
            out=p1[B, :], in0=p2, scalar1=-1.0, scalar2=168.0,
            op0=OP.mult, op1=OP.add,
        )
        q1 = sb.tile([128, 16], F32, tag="q1")
        nc.scalar.activation(out=q1[B, :], in_=p1[B, :], func=AF.Square)
        nc.vector.tensor_tensor(
            out=valcat[B, 48:64], in0=e1[B, :], in1=q1[B, :], op=OP.mult
        )
        gp = sb.tile([128, 16], F32, tag="gp")
        nc.vector.tensor_scalar(
            out=gp[B, :], in0=g2, scalar1=0.0, scalar2=None, op0=OP.max
        )
        e2 = sb.tile([128, 16], F32, tag="e2")
        nc.scalar.activation(out=e2[B, :], in_=gp[B, :], func=AF.Exp, scale=-10.0)
        q2 = sb.tile([128, 16], F32, tag="q2")
        nc.scalar.activation(out=q2[B, :], in_=p2, func=AF.Square)
        nc.vector.tensor_tensor(
            out=valcat[B, 64:80], in0=e2[B, :], in1=q2[B, :], op=OP.mult
        )

        # ---- weighted contraction: one mult + one ones-column matmul ----
        wv = sb.tile([128, NV], F32, tag="wv")
        nc.vector.tensor_tensor(
            out=wv[:], in0=valcat[:], in1=mcat_s[:], op=OP.mult
        )
        fps = ps.tile([1, NV], F32, tag="fps")
        nc.tensor.matmul(
            fps[:], ones_t[:], wv[:], start=True, stop=True,
        )
        loss_t = sb.tile([1, 1], F32, tag="loss_t")
        nc.vector.tensor_reduce(out=loss_t[:], in_=fps[:], axis=AX.X, op=OP.add)
        lossf = sb.tile([1, 1], F32, tag="lossf")
        nc.vector.tensor_scalar_add(out=lossf[:], in0=loss_t[:], scalar1=ZCONST)
        nc.sync.dma_start(out=out, in_=lossf[:])


def build_program(ndev=NCORES):
    nc = bacc.Bacc("TRN2", target_bir_lowering=False, num_devices=ndev)
    xt = nc.dram_tensor("xt", [128, L], BF16, kind="ExternalInput")
    xp8 = nc.dram_tensor("xp8", [8, L], F32, kind="ExternalInput")
    blk = nc.dram_tensor("blk", [8, 128], BF16, kind="ExternalInput")
    am8 = nc.dram_tensor("am8", [8, L], F32, kind="ExternalInput")
    wts = nc.dram_tensor("wts", [128, NK * 128 + 1], BF16, kind="ExternalInput")
    iotab = nc.dram_tensor("iotab", [128, NK], F32, kind="ExternalInput")
    mcat = nc.dram_tensor("mcat", [128, NV], F32, kind="ExternalInput")
    amtb = nc.dram_tensor("amtb", [40, L2], BF16, kind="ExternalInput")
    out = nc.dram_tensor("o", [1, 1], F32, kind="ExternalOutput")
    with tile.TileContext(nc) as tc:
        _build(
            tc, xt.ap(), xp8.ap(), blk.ap(), am8.ap(), wts.ap(),
            iotab.ap(), mcat.ap(), amtb.ap(), out.ap(),
        )
    nc.compile()
    return nc


def make_const_inputs(data):
    """Host-side constant tables shared by all cores (layout only)."""
    data = np.asarray(data, dtype=np.float32)
    # packed table [224, 16]: 5 cont cols, 7 sigma-packed binary pairs
    pk = np.zeros((224, 16), np.float32)
    pk[:223, 0:5] = data[:, 0:5]
    for u in range(7):
        pk[:223, 5 + u] = data[:, 5 + 2 * u] + SIG * data[:, 6 + 2 * u]
    pk[223, :] = pk[0, :]   # clamp target: id 223 behaves like id 0
    # 14 lhsT planes: lhsT_k[16g+j, 8c+g] = pk[16k+j, c]  (c-major out)
    wts = np.zeros((128, NK * 128 + 1), BFNP)
    for k in range(NK):
        blkm = np.zeros((128, 128), np.float32)
        for g in range(NG):
            for c in range(5):
                blkm[16 * g:16 * g + 16, 8 * c + g] = pk[16 * k:16 * k + 16, c]
            for u in range(7):
                blkm[16 * g:16 * g + 16, 64 + 8 * u + g] = (
                    pk[16 * k:16 * k + 16, 5 + u])
        wts[:, 128 * k:128 * (k + 1)] = blkm.astype(BFNP)
    wts[:, NK * 128] = np.float32(1.0)
    # iota planes: value 16k + (p % 16)
    iotab = (
        16.0 * np.arange(NK)[None, :] + (np.arange(128) % 16)[:, None]
    ).astype(np.float32)
    # broadcast selector: blk[g, p] = (p // 16 == g)  (match (g, j) layout)
    blk = (np.arange(128)[None, :] // 16 == np.arange(8)[:, None]).astype(BFNP)
    # mask-weight tile [128, NV]; c-major: row r -> c = r // 8
    r = np.arange(128)
    c = r // 8
    u = (r - 64) // 8  # binary slot index for rows 64:120
    m = np.zeros((128, NV), np.float32)
    m[:, 0:8] = (((r < 40) & (c < 5)) * W_HUB)[:, None]        # nutrition
    m[:, 8:32] = ((r < 8) * W_HUB)[:, None]                    # meal huber
    binrow = (r >= 64) & (r < 120)
    m[:, 32:40] = ((binrow & ((u == 5) | (u == 6))) * W_HUB)[:, None]
    m[:, 40:48] = ((binrow & (u >= 4) & (u <= 6)) * W_HUB)[:, None]
    m[:, 48:64] = ((binrow & (u == 0)) * W_PA)[:, None]        # prefs
    m[:, 64:72] = ((binrow & (u >= 1) & (u <= 4)) * W_PA)[:, None]
    m[:, 72:80] = ((binrow & (u >= 1) & (u <= 3)) * W_PA)[:, None]
    m[:, 80:88] = ((r < 8) / 512.0)[:, None]           # day variance
    m[0:8, 88] = -2.0 * 3000.0 / 512.0                 # tanh(2*pid) compact
    m[8:, 88] = 0.0
    m[0:8, 89] = 1.0 / 512.0                           # relu(pid-222) compact
    m[0:8, 90] = -3000.0 / 512.0                       # tanh(2*pamt) compact
    return wts, iotab, blk, m


def make_in_maps(y_pred, y, data):
    y_pred = np.asarray(y_pred, dtype=np.float32)
    y = np.asarray(y, dtype=np.float32)
    wts, iotab, blk, mcat = make_const_inputs(data)
    in_maps = []
    for core in range(NCORES):
        sl = slice(core * BL, (core + 1) * BL)

        def streams(arr, comp):
            # [64, 7, 3, 8] -> [8 streams, 1344] (batch-major within stream)
            return np.ascontiguousarray(
                arr[sl, ..., comp], dtype=np.float32).reshape(NG, L)

        pid = streams(y_pred, 0)
        pam = streams(y_pred, 1)
        tid = streams(y, 0)
        tam = streams(y, 1)
        xt = np.repeat(tid.astype(BFNP), 16, axis=0)         # [128, L] bf16
        # amounts rows 8c+g (c<5): amounts stream g; true | pred
        amtb = np.tile(
            np.concatenate([tam, pam], axis=1), (5, 1)).astype(BFNP)
        in_maps.append({
            "xt": xt, "xp8": pid, "blk": blk, "am8": pam,
            "wts": wts, "iotab": iotab, "mcat": mcat, "amtb": amtb,
        })
    return in_maps


_NC_CACHE = None


def _get_nc():
    global _NC_CACHE
    if _NC_CACHE is None:
        _NC_CACHE = build_program()
    return _NC_CACHE


def run_on_hw(y_pred, y, data, **kwargs):
    from concourse.bass_utils import run_bass_kernel_spmd

    nc = _get_nc()
    in_maps = make_in_maps(y_pred, y, data)
    res = run_bass_kernel_spmd(
        nc, in_maps, core_ids=list(range(NCORES)), **kwargs
    )
    parts = [r["o"][0, 0] for r in res.results]
    return np.float32(np.sum(np.asarray(parts, dtype=np.float32))), res


def kernel(y_pred, y, data):
    return run_on_hw(y_pred, y, data)[0]
